# revision 1
# baseline (speedup 1.0000x reference)
"""Trainium2 Bass kernel for causal multi-head attention (B=2, T=2048, D=1024, H=16).

Sharding: 8 cores = 2 batches x 4 head-groups. Each core computes 4 heads
(as 2 head-pairs packed into 128 partitions) of one batch, plus its row-shard
of the output projection; the host sums the 4 partial outputs per batch.

Compute is bf16 with fp32 PSUM accumulation. Softmax uses no max-subtraction
(scores ~ N(0,1), exp is safe in fp32) and gets the denominator for free via
an all-ones column appended to V.
"""

import numpy as np
import ml_dtypes
from contextlib import ExitStack

import concourse.bass as bass
import concourse.mybir as mybir
import concourse.tile as tile
from concourse import bacc
from concourse.bass_utils import run_bass_kernel_spmd

BF16 = mybir.dt.bfloat16
F32 = mybir.dt.float32
AF = mybir.ActivationFunctionType
bf16 = ml_dtypes.bfloat16

B, T, D, H, DH = 2, 2048, 1024, 16, 64
NCORES = 8
QTILE = 512          # q columns per score tile
NQT = T // QTILE     # 4
TCH = T // 128       # 16 t-chunks / k-blocks

_CACHE = {}


def _build():
    nc = bacc.Bacc(
        "TRN2", target_bir_lowering=False, debug=False, num_devices=NCORES
    )
    # wb columns: [wq0|wq1|wk0|wk1|wv0|wv1|msk] (7x128), one DMA
    xt_d = nc.dram_tensor("xt", [256, T], BF16, kind="ExternalInput").ap()
    wb_d = nc.dram_tensor("wb", [128, 896], BF16, kind="ExternalInput").ap()
    wo_d = nc.dram_tensor("wo", [128, 2 * D], BF16, kind="ExternalInput").ap()
    bias_d = nc.dram_tensor("bias", [128, 4], F32, kind="ExternalInput").ap()
    y_d = nc.dram_tensor("y", [T, D], BF16, kind="ExternalOutput").ap()

    with tile.TileContext(nc) as tc, ExitStack() as ctx:
        const = ctx.enter_context(tc.tile_pool(name="const", bufs=1))
        pers = ctx.enter_context(tc.tile_pool(name="pers", bufs=1))
        pex = ctx.enter_context(tc.tile_pool(name="pex", bufs=2))
        patt = ctx.enter_context(tc.tile_pool(name="patt", bufs=4))
        pysb = ctx.enter_context(tc.tile_pool(name="pysb", bufs=2))
        ps_s = ctx.enter_context(tc.tile_pool(name="ps_s", bufs=2, space="PSUM"))
        ps_a = ctx.enter_context(tc.tile_pool(name="ps_a", bufs=2, space="PSUM"))
        ps_m = ctx.enter_context(tc.tile_pool(name="ps_m", bufs=2, space="PSUM"))

        # Consolidated input loads: each dma_start costs ~1us of serial queue
        # setup, so the small tensors ride in one blob. The first projection
        # chunk's inputs go first on their queues.
        wb_sb = const.tile([128, 896], BF16, tag="wb", name="wb_sb")
        nc.scalar.dma_start(wb_sb[:], wb_d)
        bias_sb = const.tile([128, 4], F32, tag="bias", name="bias_sb")
        nc.scalar.dma_start(bias_sb[:], bias_d)
        wq_sb = [wb_sb[:, 128 * p : 128 * (p + 1)] for p in range(2)]
        wk_sb = [wb_sb[:, 256 + 128 * p : 256 + 128 * (p + 1)] for p in range(2)]
        wv_sb = [wb_sb[:, 512 + 128 * p : 512 + 128 * (p + 1)] for p in range(2)]
        msk_sb = wb_sb[:, 768:896]
        bq_sb = [bias_sb[:, p : p + 1] for p in range(2)]
        bk_sb = [bias_sb[:, 2 + p : 3 + p] for p in range(2)]

        xt_sb, qT, kT, vaug, attnT = [], [], [], [], []
        for p in range(2):
            eng = nc.sync if p == 0 else nc.gpsimd
            t_ = pers.tile([128, T], BF16, tag=f"xt{p}", name=f"xt{p}_sb")
            for j in range(4):
                sl = slice(QTILE * j, QTILE * (j + 1))
                eng.dma_start(t_[:, sl], xt_d[128 * p : 128 * (p + 1), sl])
            xt_sb.append(t_)

        wof_sb = const.tile([128, 2 * D], BF16, tag="wo", name="wof_sb")
        nc.gpsimd.dma_start(wof_sb[:], wo_d)
        wo_sb = [wof_sb[:, D * p : D * (p + 1)] for p in range(2)]

        for p in range(2):
            qT.append(pers.tile([128, T], BF16, tag=f"qT{p}", name=f"qT{p}_sb"))
            kT.append(pers.tile([128, T], BF16, tag=f"kT{p}", name=f"kT{p}_sb"))
            vaug.append(
                pers.tile([128, 256 * TCH], BF16, tag=f"va{p}", name=f"va{p}_sb")
            )
            attnT.append(
                pers.tile([128, T], BF16, tag=f"aT{p}", name=f"aT{p}_sb")
            )

        # ---- Phase A: QKV projections (2-head block-diagonal packing) ----
        for p in range(2):
            for j in range(NQT):
                sl = slice(QTILE * j, QTILE * (j + 1))
                pq = ps_m.tile([128, QTILE], F32, tag="m", name="pq")
                nc.tensor.matmul(
                    pq[:], wq_sb[p][:], xt_sb[p][:, sl], start=True, stop=True
                )
                nc.scalar.activation(
                    qT[p][:, sl], pq[:], AF.Identity, bias=bq_sb[p][:]
                )
                pk = ps_m.tile([128, QTILE], F32, tag="m", name="pk")
                nc.tensor.matmul(
                    pk[:], wk_sb[p][:], xt_sb[p][:, sl], start=True, stop=True
                )
                nc.scalar.activation(
                    kT[p][:, sl], pk[:], AF.Identity, bias=bk_sb[p][:]
                )
            # V_aug layout per (t-chunk, head): [V_h | ones x 64] (M=128 each).
            # The ones half makes the AV^T matmul emit the softmax denominator
            # replicated across 64 partitions, rows 64:128 of its output.
            nc.vector.memset(
                vaug[p].rearrange("p (g c) -> p g c", c=128)[:, :, 64:128], 1.0
            )
            for tj in range(TCH):
                pv = ps_m.tile([128, QTILE], F32, tag="m", name="pv")
                nc.tensor.matmul(
                    pv[:, 0:128],
                    xt_sb[p][:, 128 * tj : 128 * (tj + 1)],
                    wv_sb[p][:],
                    start=True,
                    stop=True,
                )
                src = pv[:, 0:128].rearrange("p (h x) -> p h x", h=2)
                dst = vaug[p][:, 256 * tj : 256 * tj + 256].rearrange(
                    "p (h c) -> p h c", h=2
                )[:, :, 0:64]
                nc.vector.tensor_copy(dst, src)

        # ---- Phase B: attention + output projection ----
        def avt(p, h, qi, kb, aps, es_t):
            # out^T = [V | 1]^T @ es accumulated over k-blocks: rows 0:64 are
            # attn@V transposed, rows 64:128 the softmax denominator
            # replicated 64x (free broadcast for the division).
            nkb = 4 * (qi + 1)
            cs = max(0, 128 * (kb - 4 * qi))
            nc.tensor.matmul(
                aps[:, cs:QTILE],
                vaug[p][:, 256 * kb + 128 * h : 256 * kb + 128 * (h + 1)],
                es_t[:, 1024 * kb + 512 * h + cs : 1024 * kb + 512 * (h + 1)],
                start=(kb == 0),
                stop=(kb == nkb - 1),
            )

        def outproj_tj(tj):
            # output projection for one 128-row t-chunk
            tsl = slice(128 * tj, 128 * (tj + 1))
            y0 = ps_m.tile([128, 512], F32, tag="m", name="y0")
            y1 = ps_m.tile([128, 512], F32, tag="m", name="y1")
            for half, yp in ((0, y0), (1, y1)):
                nsl = slice(512 * half, 512 * (half + 1))
                nc.tensor.matmul(
                    yp[:], attnT[0][:, tsl], wo_sb[0][:, nsl],
                    start=True, stop=False,
                )
                nc.tensor.matmul(
                    yp[:], attnT[1][:, tsl], wo_sb[1][:, nsl],
                    start=False, stop=True,
                )
            ysb = pysb.tile([128, D], BF16, tag="y", name="ysb")
            nc.vector.tensor_copy(ysb[:, 0:512], y0[:])
            nc.vector.tensor_copy(ysb[:, 512:1024], y1[:])
            nc.sync.dma_start(y_d[tsl, :], ysb[:])

        qi_order = list(range(NQT))
        for iq, qi in enumerate(qi_order):
            q0 = QTILE * qi
            nkb = 4 * (qi + 1)
            for p in range(2):
                es_t = pex.tile([128, 1024 * nkb], BF16, tag="es", name="es_t")
                aps = [
                    ps_a.tile([128, QTILE], F32, tag="a", name=f"aps{h}")
                    for h in range(2)
                ]
                # Per k-block: previous block's AV^T pair first, then the
                # row-packed scores pair back-to-back (keeps them concurrent
                # in the array), then the joint exp + causal mask strip.
                # The k-loop is exp-gated on ScalarE, so the previous q-tile's
                # output-projection chunks are sprinkled in to fill PE slack.
                for kb in range(nkb):
                    cs = max(0, 128 * (kb - 4 * qi))
                    k0 = 128 * kb
                    if kb > 0:
                        for h in range(2):
                            avt(p, h, qi, kb - 1, aps[h], es_t)
                    sps = ps_s.tile([128, 1024], F32, tag="s", name="sps")
                    with tc.tile_critical():
                        # keep the row-packed pair adjacent so both halves of
                        # the PE array stream the two heads concurrently
                        for h in range(2):
                            hp = slice(64 * h, 64 * (h + 1))
                            nc.tensor.matmul(
                                sps[:, 512 * h + cs : 512 * (h + 1)],
                                kT[p][hp, k0 : k0 + 128],
                                qT[p][hp, q0 + cs : q0 + QTILE],
                                start=True,
                                stop=True,
                            )
                    nc.scalar.activation(
                        es_t[:, 1024 * kb : 1024 * (kb + 1)].rearrange(
                            "p (h x) -> p h x", h=2
                        )[:, :, cs:512],
                        sps.rearrange("p (h x) -> p h x", h=2)[:, :, cs:512],
                        AF.Exp,
                    )
                    if kb >= 4 * qi:  # diagonal block: mask the 128-strips
                        for h in range(2):
                            stp = slice(
                                1024 * kb + 512 * h + cs,
                                1024 * kb + 512 * h + cs + 128,
                            )
                            nc.gpsimd.tensor_mul(
                                es_t[:, stp], es_t[:, stp], msk_sb[:]
                            )
                if p == 0 and iq > 0:
                    prev = qi_order[iq - 1]
                    for tj in range(4 * prev, 4 * (prev + 1)):
                        outproj_tj(tj)  # deferred: divisions are long done
                for h in range(2):
                    avt(p, h, qi, nkb - 1, aps[h], es_t)
                for h in range(2):
                    # custom-DVE recip ucode ignores base_partition 64 on HW,
                    # so shift the sums to base 0 with a plain copy first
                    sms = patt.tile([64, QTILE], F32, tag="sms", name="sms")
                    rec = patt.tile([64, QTILE], F32, tag="rec", name="rec")
                    nc.vector.tensor_copy(sms[:], aps[h][64:128, :])
                    nc.vector.reciprocal_approx_fast(rec[:], sms[:])
                    nc.vector.tensor_mul(
                        attnT[p][64 * h : 64 * (h + 1), q0 : q0 + QTILE],
                        aps[h][0:64, :],
                        rec[:],
                    )
        last = qi_order[-1]
        for tj in range(4 * last, 4 * (last + 1)):
            outproj_tj(tj)

    nc.compile()
    return nc


def _host_prep(x, Wq, bq, Wk, bk, Wv, bv, Wo, bo):
    x = np.asarray(x, np.float32)
    Wq, bq = np.asarray(Wq, np.float32), np.asarray(bq, np.float32)
    Wk, bk = np.asarray(Wk, np.float32), np.asarray(bk, np.float32)
    Wv, bv = np.asarray(Wv, np.float32), np.asarray(bv, np.float32)
    Wo, bo = np.asarray(Wo, np.float32), np.asarray(bo, np.float32)
    msk = np.triu(np.ones((128, 128), np.float32))
    in_maps = []
    for c in range(NCORES):
        b, g = divmod(c, 4)
        h0 = 4 * g
        xt = np.ascontiguousarray(x[b, :, 256 * g : 256 * (g + 1)].T).astype(bf16)
        wb = np.zeros((128, 896), np.float32)
        bias = np.zeros((128, 4), np.float32)
        for p in range(2):
            ha, hb = h0 + 2 * p, h0 + 2 * p + 1
            wb[0:64, 128 * p : 128 * p + 64] = Wq[ha] * 0.125
            wb[64:128, 128 * p + 64 : 128 * p + 128] = Wq[hb] * 0.125
            wb[0:64, 256 + 128 * p : 256 + 128 * p + 64] = Wk[ha]
            wb[64:128, 256 + 128 * p + 64 : 256 + 128 * p + 128] = Wk[hb]
            wb[0:64, 512 + 128 * p : 512 + 128 * p + 64] = Wv[ha]
            wb[64:128, 512 + 128 * p + 64 : 512 + 128 * p + 128] = Wv[hb]
            bias[0:64, p] = bq[ha] * 0.125
            bias[64:128, p] = bq[hb] * 0.125
            bias[0:64, 2 + p] = bk[ha]
            bias[64:128, 2 + p] = bk[hb]
        wb[:, 768:896] = msk
        wo_c = np.ascontiguousarray(
            Wo[256 * g : 256 * (g + 1)].reshape(2, 128, D).transpose(1, 0, 2)
            .reshape(128, 2 * D)
        ).astype(bf16)
        in_maps.append(
            {"xt": xt, "wb": wb.astype(bf16), "wo": wo_c, "bias": bias}
        )
    # bv contributes bv_flat @ Wo to every output row (softmax weights sum to 1)
    bo_eff = bo + bv.reshape(-1) @ Wo
    return in_maps, bo_eff


def _finalize(results, bo_eff):
    out = np.zeros((B, T, D), np.float32)
    for c in range(NCORES):
        out[c // 4] += np.asarray(results[c]["y"], dtype=np.float32)
    out += bo_eff[None, None, :]
    return out


def kernel(**inputs):
    if "nc" not in _CACHE:
        _CACHE["nc"] = _build()
    nc = _CACHE["nc"]
    in_maps, bo_eff = _host_prep(**inputs)
    res = run_bass_kernel_spmd(
        nc, in_maps, core_ids=list(range(NCORES)), trace=False
    )
    return _finalize(res.results, bo_eff)


def kernel_traced(**inputs):
    """Dev helper: run with NTFF profiling, return (out, exec_time_ns, tmpdir)."""
    import glob
    import tempfile

    from concourse import bass2jax
    from trn_agent_boot.trn_boot import _ntff_profile_via_ctypes

    if "nc" not in _CACHE:
        _CACHE["nc"] = _build()
    nc = _CACHE["nc"]
    in_maps, bo_eff = _host_prep(**inputs)
    hook = _ntff_profile_via_ctypes("/opt/axon/libaxon_pjrt.so")
    tmpdir = tempfile.mkdtemp(prefix="mha_trace_")
    with hook(tmpdir, [0]):
        results = bass2jax.run_bass_via_pjrt(nc, in_maps, n_cores=NCORES)
    out = _finalize(results, bo_eff)

    exec_time_ns = None
    try:
        import gauge.profiler
        from concourse._compat import FishPath

        ntffs = glob.glob(f"{tmpdir}/*.ntff")
        if ntffs:
            profile = gauge.profiler.Profile(
                profile_path=FishPath(tmpdir),
                kernel_dev_mode=True,
                profile_on_exit=False,
                bass_kernel=nc.m,
                offline_processing=True,
                fname="*_body*",
            )
            pres = profile.to_perfetto(model_index=(0,))
            if pres:
                exec_time_ns = pres[0].exec_time_ns
    except Exception as e:  # profiling is best-effort
        print(f"profile processing failed: {type(e).__name__}: {e}")
    return out, exec_time_ns, tmpdir



# revision 2
# speedup vs baseline: 2.0504x; 2.0504x over previous
"""Trainium2 Bass kernel for causal multi-head attention (B=2, T=2048, D=1024, H=16).

Sharding: 8 cores = 2 batches x 4 head-groups. Each core computes 4 heads
(as 2 head-pairs packed into 128 partitions) of one batch, plus its row-shard
of the output projection; the host sums the 4 partial outputs per batch.

Compute is bf16 with fp32 PSUM accumulation. Softmax uses no max-subtraction
(scores ~ N(0,1), exp is safe in fp32) and gets the denominator for free via
an all-ones column block prepended to V.

Pipeline design (PE p-state aware): the TRN2 PE only reaches full clock after
3us of continuous execution, so the tensor queue must never stall. The causal
mask is additive (-87), folded into the scores PSUM by a small matmul
(mask^T stationary, identity streaming) so no other engine sits between the
exp and the AV^T matmul. AV^T lags the scores by 2 k-blocks so the exp
latency (ScalarE) is fully hidden. ScalarE runs only exp; Q/K bias-adds and
all PSUM->SBUF casts live on VectorE. Output-projection chunks for q-tile i
are sprinkled into q-tile i+1's k-loop as PE filler.
"""

import numpy as np
import ml_dtypes
from contextlib import ExitStack

import concourse.bass as bass
import concourse.mybir as mybir
import concourse.tile as tile
from concourse import bacc
from concourse.bass_utils import run_bass_kernel_spmd

BF16 = mybir.dt.bfloat16
F32 = mybir.dt.float32
AF = mybir.ActivationFunctionType
bf16 = ml_dtypes.bfloat16

B, T, D, H, DH = 2, 2048, 1024, 16, 64
NCORES = 8
QTILE = 512          # q columns per score tile
NQT = T // QTILE     # 4
TCH = T // 128       # 16 t-chunks / k-blocks
MASKVAL = -87.0      # additive causal mask; exp(-87 + s) ~ 0 in fp32

_CACHE = {}


def _build():
    nc = bacc.Bacc(
        "TRN2", target_bir_lowering=False, debug=False, num_devices=NCORES
    )
    # wb columns: [wq0|wq1|wk0|wk1|wv0|wv1|maskT|ident] (8x128), one DMA
    xt_d = nc.dram_tensor("xt", [256, T], BF16, kind="ExternalInput").ap()
    wb_d = nc.dram_tensor("wb", [128, 1024], BF16, kind="ExternalInput").ap()
    wo_d = nc.dram_tensor("wo", [128, 2 * D], BF16, kind="ExternalInput").ap()
    bias_d = nc.dram_tensor("bias", [128, 4], F32, kind="ExternalInput").ap()
    y_d = nc.dram_tensor("y", [T, D], BF16, kind="ExternalOutput").ap()

    with tile.TileContext(nc) as tc, ExitStack() as ctx:
        const = ctx.enter_context(tc.tile_pool(name="const", bufs=1))
        pers = ctx.enter_context(tc.tile_pool(name="pers", bufs=1))
        pex = ctx.enter_context(tc.tile_pool(name="pex", bufs=2))
        pdiv = ctx.enter_context(tc.tile_pool(name="pdiv", bufs=4))
        pysb = ctx.enter_context(tc.tile_pool(name="pysb", bufs=2))
        ps_s = ctx.enter_context(tc.tile_pool(name="ps_s", bufs=2, space="PSUM"))
        ps_a = ctx.enter_context(tc.tile_pool(name="ps_a", bufs=2, space="PSUM"))
        ps_m = ctx.enter_context(tc.tile_pool(name="ps_m", bufs=2, space="PSUM"))

        # Consolidated input loads. gpsimd issues DMAs nearly for free
        # (hardware DGE); sync carries the other xt half + output stores.
        wb_sb = const.tile([128, 1024], BF16, tag="wb", name="wb_sb")
        nc.gpsimd.dma_start(wb_sb[:], wb_d)
        bias_sb = const.tile([128, 4], F32, tag="bias", name="bias_sb")
        nc.gpsimd.dma_start(bias_sb[:], bias_d)
        wq_sb = [wb_sb[:, 128 * p : 128 * (p + 1)] for p in range(2)]
        wk_sb = [wb_sb[:, 256 + 128 * p : 256 + 128 * (p + 1)] for p in range(2)]
        wv_sb = [wb_sb[:, 512 + 128 * p : 512 + 128 * (p + 1)] for p in range(2)]
        msk_sb = wb_sb[:, 768:896]
        idn_sb = wb_sb[:, 896:1024]
        bq_sb = [bias_sb[:, p : p + 1] for p in range(2)]
        bk_sb = [bias_sb[:, 2 + p : 3 + p] for p in range(2)]

        xt_sb = []
        for p in range(2):
            eng = nc.sync if p == 0 else nc.gpsimd
            t_ = pers.tile([128, T], BF16, tag=f"xt{p}", name=f"xt{p}_sb")
            for j in range(4):
                sl = slice(QTILE * j, QTILE * (j + 1))
                eng.dma_start(t_[:, sl], xt_d[128 * p : 128 * (p + 1), sl])
            xt_sb.append(t_)

        wof_sb = const.tile([128, 2 * D], BF16, tag="wo", name="wof_sb")
        nc.gpsimd.dma_start(wof_sb[:], wo_d)
        wo_sb = [wof_sb[:, D * p : D * (p + 1)] for p in range(2)]

        qT, kT, vaug, attnT = [], [], [], []
        for p in range(2):
            qT.append(pers.tile([128, T], BF16, tag=f"qT{p}", name=f"qT{p}_sb"))
            kT.append(pers.tile([128, T], BF16, tag=f"kT{p}", name=f"kT{p}_sb"))
            vaug.append(
                pers.tile([128, 256 * TCH], BF16, tag=f"va{p}", name=f"va{p}_sb")
            )
            attnT.append(
                pers.tile([128, T], BF16, tag=f"aT{p}", name=f"aT{p}_sb")
            )

        # ---- Phase A: QKV projections (2-head block-diagonal packing) ----
        def phase_a(p):
            for j in range(NQT):
                sl = slice(QTILE * j, QTILE * (j + 1))
                pq = ps_m.tile([128, QTILE], F32, tag="m", name="pq")
                nc.tensor.matmul(
                    pq[:], wq_sb[p][:], xt_sb[p][:, sl], start=True, stop=True
                )
                nc.vector.tensor_scalar_add(qT[p][:, sl], pq[:], bq_sb[p][:])
                pk = ps_m.tile([128, QTILE], F32, tag="m", name="pk")
                nc.tensor.matmul(
                    pk[:], wk_sb[p][:], xt_sb[p][:, sl], start=True, stop=True
                )
                nc.vector.tensor_scalar_add(kT[p][:, sl], pk[:], bk_sb[p][:])
            # V_aug layout per (t-chunk, head): [ones x 64 | V_h] (M=128 each).
            # The ones half makes the AV^T matmul emit the softmax denominator
            # replicated across partitions 0:64 of its output (where the
            # custom-DVE reciprocal can read it directly).
            nc.gpsimd.memset(
                vaug[p].rearrange("p (g c) -> p g c", c=128)[:, :, 0:64], 1.0
            )
            for c4 in range(4):  # 4 t-chunks per V psum tile
                pv = ps_m.tile([128, QTILE], F32, tag="m", name="pv")
                for j in range(4):
                    tj = 4 * c4 + j
                    nc.tensor.matmul(
                        pv[:, 128 * j : 128 * (j + 1)],
                        xt_sb[p][:, 128 * tj : 128 * (tj + 1)],
                        wv_sb[p][:],
                        start=True,
                        stop=True,
                    )
                src = pv.rearrange("p (c h x) -> p c h x", c=4, h=2)
                dst = vaug[p][:, 1024 * c4 : 1024 * (c4 + 1)].rearrange(
                    "p (c h x) -> p c h x", c=4, h=2
                )[:, :, :, 64:128]
                nc.vector.tensor_copy(dst, src)

        # ---- Phase B: attention + output projection ----
        def avt_pair(p, qi, kb, nkb, aps, es_t):
            # out^T = [1 | V]^T @ es accumulated over k-blocks: rows 0:64 are
            # the softmax denominator replicated 64x, rows 64:128 attn@V
            # transposed.
            cs = max(0, 128 * (kb - 4 * qi))
            for h in range(2):
                nc.tensor.matmul(
                    aps[h][:, cs:QTILE],
                    vaug[p][:, 256 * kb + 128 * h : 256 * kb + 128 * (h + 1)],
                    es_t[:, 1024 * kb + 512 * h + cs : 1024 * kb + 512 * (h + 1)],
                    start=(kb == 0),
                    stop=(kb == nkb - 1),
                )

        def outproj_tj(tj):
            # output projection for one 128-row t-chunk
            tsl = slice(128 * tj, 128 * (tj + 1))
            y0 = ps_m.tile([128, 512], F32, tag="m", name="y0")
            y1 = ps_m.tile([128, 512], F32, tag="m", name="y1")
            for half, yp in ((0, y0), (1, y1)):
                nsl = slice(512 * half, 512 * (half + 1))
                nc.tensor.matmul(
                    yp[:], attnT[0][:, tsl], wo_sb[0][:, nsl],
                    start=True, stop=False,
                )
                nc.tensor.matmul(
                    yp[:], attnT[1][:, tsl], wo_sb[1][:, nsl],
                    start=False, stop=True,
                )
            ysb = pysb.tile([128, D], BF16, tag="y", name="ysb")
            nc.vector.tensor_copy(ysb[:, 0:512], y0[:])
            nc.vector.tensor_copy(ysb[:, 512:1024], y1[:])
            nc.sync.dma_start(y_d[tsl, :], ysb[:])

        def group(p, qi, mid_tj, tail_tj):
            # mid_tj: {kb: tj} outproj chunks emitted inside the k-loop as PE
            # filler; tail_tj: outproj chunk emitted between the two tail AV^T
            # pairs (covers the last exp's latency).
            q0 = QTILE * qi
            nkb = 4 * (qi + 1)
            es_t = pex.tile([128, 1024 * nkb], BF16, tag="es", name="es_t")
            aps = [
                ps_a.tile([128, QTILE], F32, tag="a", name=f"aps{h}")
                for h in range(2)
            ]
            for kb in range(nkb):
                cs = max(0, 128 * (kb - 4 * qi))
                k0 = 128 * kb
                sps = ps_s.tile([128, 1024], F32, tag="s", name="sps")
                for h in range(2):
                    hp = slice(64 * h, 64 * (h + 1))
                    nc.tensor.matmul(
                        sps[:, 512 * h + cs : 512 * (h + 1)],
                        kT[p][hp, k0 : k0 + 128],
                        qT[p][hp, q0 + cs : q0 + QTILE],
                        start=True,
                        stop=True,
                    )
                if kb >= 4 * qi:  # diagonal block: add -87 above the diagonal
                    for h in range(2):
                        nc.tensor.matmul(
                            sps[:, 512 * h + cs : 512 * h + cs + 128],
                            msk_sb,
                            idn_sb,
                            start=False,
                            stop=True,
                        )
                nc.scalar.activation(
                    es_t[:, 1024 * kb : 1024 * (kb + 1)].rearrange(
                        "p (h x) -> p h x", h=2
                    )[:, :, cs:512],
                    sps.rearrange("p (h x) -> p h x", h=2)[:, :, cs:512],
                    AF.Exp,
                )
                if kb >= 2:
                    avt_pair(p, qi, kb - 2, nkb, aps, es_t)
                if kb in mid_tj:
                    outproj_tj(mid_tj[kb])
            avt_pair(p, qi, nkb - 2, nkb, aps, es_t)
            if tail_tj is not None:
                outproj_tj(tail_tj)
            avt_pair(p, qi, nkb - 1, nkb, aps, es_t)
            for h in range(2):
                rec = pdiv.tile([64, QTILE], F32, tag="rec", name="rec")
                nc.vector.reciprocal_approx_fast(rec[:], aps[h][0:64, :])
                nc.vector.tensor_mul(
                    attnT[p][64 * h : 64 * (h + 1), q0 : q0 + QTILE],
                    aps[h][64:128, :],
                    rec[:],
                )

        phase_a(0)
        group(0, 0, {}, None)
        phase_a(1)
        group(1, 0, {}, None)
        for qi in range(1, NQT):
            t0 = 4 * (qi - 1)
            group(0, qi, {2: t0 + 0, 4: t0 + 1}, t0 + 2)
            group(1, qi, {}, t0 + 3)
        for tj in range(4 * (NQT - 1), 4 * NQT):
            outproj_tj(tj)

    nc.compile()
    return nc


def _host_prep(x, Wq, bq, Wk, bk, Wv, bv, Wo, bo):
    x = np.asarray(x, np.float32)
    Wq, bq = np.asarray(Wq, np.float32), np.asarray(bq, np.float32)
    Wk, bk = np.asarray(Wk, np.float32), np.asarray(bk, np.float32)
    Wv, bv = np.asarray(Wv, np.float32), np.asarray(bv, np.float32)
    Wo, bo = np.asarray(Wo, np.float32), np.asarray(bo, np.float32)
    # additive-mask matmul: psum[i,j] += sum_p lhsT[p,i]*I[p,j] = lhsT[j,i],
    # want MASKVAL where k>q i.e. i>j  =>  lhsT = MASKVAL*triu(ones, 1)
    mskT = MASKVAL * np.triu(np.ones((128, 128), np.float32), 1)
    ident = np.eye(128, dtype=np.float32)
    in_maps = []
    for c in range(NCORES):
        b, g = divmod(c, 4)
        h0 = 4 * g
        xt = np.ascontiguousarray(x[b, :, 256 * g : 256 * (g + 1)].T).astype(bf16)
        wb = np.zeros((128, 1024), np.float32)
        bias = np.zeros((128, 4), np.float32)
        for p in range(2):
            ha, hb = h0 + 2 * p, h0 + 2 * p + 1
            wb[0:64, 128 * p : 128 * p + 64] = Wq[ha] * 0.125
            wb[64:128, 128 * p + 64 : 128 * p + 128] = Wq[hb] * 0.125
            wb[0:64, 256 + 128 * p : 256 + 128 * p + 64] = Wk[ha]
            wb[64:128, 256 + 128 * p + 64 : 256 + 128 * p + 128] = Wk[hb]
            wb[0:64, 512 + 128 * p : 512 + 128 * p + 64] = Wv[ha]
            wb[64:128, 512 + 128 * p + 64 : 512 + 128 * p + 128] = Wv[hb]
            bias[0:64, p] = bq[ha] * 0.125
            bias[64:128, p] = bq[hb] * 0.125
            bias[0:64, 2 + p] = bk[ha]
            bias[64:128, 2 + p] = bk[hb]
        wb[:, 768:896] = mskT
        wb[:, 896:1024] = ident
        wo_c = np.ascontiguousarray(
            Wo[256 * g : 256 * (g + 1)].reshape(2, 128, D).transpose(1, 0, 2)
            .reshape(128, 2 * D)
        ).astype(bf16)
        in_maps.append(
            {"xt": xt, "wb": wb.astype(bf16), "wo": wo_c, "bias": bias}
        )
    # bv contributes bv_flat @ Wo to every output row (softmax weights sum to 1)
    bo_eff = bo + bv.reshape(-1) @ Wo
    return in_maps, bo_eff


def _finalize(results, bo_eff):
    out = np.zeros((B, T, D), np.float32)
    for c in range(NCORES):
        out[c // 4] += np.asarray(results[c]["y"], dtype=np.float32)
    out += bo_eff[None, None, :]
    return out


def kernel(**inputs):
    if "nc" not in _CACHE:
        _CACHE["nc"] = _build()
    nc = _CACHE["nc"]
    in_maps, bo_eff = _host_prep(**inputs)
    res = run_bass_kernel_spmd(
        nc, in_maps, core_ids=list(range(NCORES)), trace=False
    )
    return _finalize(res.results, bo_eff)


def kernel_traced(**inputs):
    """Dev helper: run with NTFF profiling, return (out, exec_time_ns, tmpdir)."""
    import glob
    import tempfile

    from concourse import bass2jax
    from trn_agent_boot.trn_boot import _ntff_profile_via_ctypes

    if "nc" not in _CACHE:
        _CACHE["nc"] = _build()
    nc = _CACHE["nc"]
    in_maps, bo_eff = _host_prep(**inputs)
    hook = _ntff_profile_via_ctypes("/opt/axon/libaxon_pjrt.so")
    tmpdir = tempfile.mkdtemp(prefix="mha_trace_")
    with hook(tmpdir, [0]):
        results = bass2jax.run_bass_via_pjrt(nc, in_maps, n_cores=NCORES)
    out = _finalize(results, bo_eff)

    exec_time_ns = None
    try:
        import gauge.profiler
        from concourse._compat import FishPath

        ntffs = glob.glob(f"{tmpdir}/*.ntff")
        if ntffs:
            profile = gauge.profiler.Profile(
                profile_path=FishPath(tmpdir),
                kernel_dev_mode=True,
                profile_on_exit=False,
                bass_kernel=nc.m,
                offline_processing=True,
                fname="*_body*",
            )
            pres = profile.to_perfetto(model_index=(0,))
            if pres:
                exec_time_ns = pres[0].exec_time_ns
    except Exception as e:  # profiling is best-effort
        print(f"profile processing failed: {type(e).__name__}: {e}")
    return out, exec_time_ns, tmpdir
